# revision 8
# baseline (speedup 1.0000x reference)
"""Trainium2 Bass kernel: 2-layer bidirectional GRU decoder + dense/softmax head.

Data-parallel over 8 NeuronCores (batch 4096 -> 512 per core). Inside each
core everything runs transposed: partition dim = hidden units with
[fwd(64); bwd(64)] packed to 128 partitions, batch on the free dim.
"""

import os
import sys

sys.path.insert(0, "/opt/trn_rl_repo")

from contextlib import ExitStack

import numpy as np

import concourse.bass as bass
import concourse.bacc as bacc
import concourse.tile as tile
from concourse import mybir
from concourse.bass_utils import run_bass_kernel_spmd

AF = mybir.ActivationFunctionType
OP = mybir.AluOpType
DT = mybir.dt

B, T, F, H, DENSE, P = 4096, 72, 9, 64, 128, 24
NCORES = 8
BC = B // NCORES  # 512 batch per core
G3 = 3 * H

# ---- tuning knobs ----
N_CHUNK = 256        # batch columns per chain (512 = 1 chain, 256 = 2 chains)
STREAM_DT = "bf16"    # "f32" (fp32 storage, fp32r matmuls) or "bf16"
MM_EXACT = False     # True: plain fp32 matmuls (4 cyc/row) for max accuracy
SPLIT_SIG = False    # True: two [128,N] sigmoids (r first); False: one [128,2N]
NO_INJECT = False    # True: s = xh + t on DVE; False: PE identity-inject


def _np_dt():
    return np.float32 if STREAM_DT == "f32" else np.dtype("bfloat16")


def _mb_dt():
    return DT.float32 if STREAM_DT == "f32" else DT.bfloat16


def _mm(ap):
    """Cast an AP to the matmul dtype (fp32r trick for fp32 streams)."""
    if STREAM_DT == "f32" and not MM_EXACT:
        return ap.bitcast(DT.float32r)
    return ap


WEIGHT_NAMES = (
    ["l1x_z", "l1x_r", "l1x_h"]
    + ["l1u_z", "l1u_r", "l1u_h"]
    + ["l2a_z", "l2a_r", "l2a_h"]
    + ["l2b_z", "l2b_r", "l2b_h"]
    + ["l2u_z", "l2u_r", "l2u_h"]
    + ["ident", "dense_w", "out_w"]
)
VEC_NAMES = ["br1h", "br2h", "bi2h", "bz2", "br2", "dense_b", "out_b",
             "ones_a", "ones_b"]


def build_module(split_l2_sigmoid: bool, reps: int = 1):
    nc = bacc.Bacc("TRN2", target_bir_lowering=False, debug=False)
    sdt = _mb_dt()

    # ---- DRAM I/O ----
    d_xin = nc.dram_tensor("xin", [2 * F + 1, T * BC], sdt, kind="ExternalInput").ap()
    d_h0 = nc.dram_tensor("h0", [2 * H, BC], sdt, kind="ExternalInput").ap()
    d_w = {}
    for n in WEIGHT_NAMES:
        shape = {
            "l1x_z": [2 * F + 1, 2 * H], "l1x_r": [2 * F + 1, 2 * H],
            "l1x_h": [2 * F + 1, 2 * H],
            "dense_w": [2 * H, DENSE], "out_w": [DENSE, P],
        }.get(n, [2 * H, 2 * H])
        d_w[n] = nc.dram_tensor(n, shape, sdt, kind="ExternalInput").ap()
    d_v = {}
    for n in VEC_NAMES:
        shape = {"out_b": [P, 1], "ones_a": [P, 1], "ones_b": [1, P]}.get(n, [2 * H, 1])
        d_v[n] = nc.dram_tensor(n, shape, DT.float32, kind="ExternalInput").ap()
    d_out = nc.dram_tensor("out", [P, BC], DT.float32, kind="ExternalOutput").ap()

    N = N_CHUNK
    NCH = BC // N  # number of chains

    with tile.TileContext(nc) as tc, ExitStack() as ctx:
        wpool = ctx.enter_context(tc.tile_pool(name="weights", bufs=1))
        seq_pool = ctx.enter_context(tc.tile_pool(name="seq", bufs=1))
        spool = ctx.enter_context(tc.tile_pool(name="sig", bufs=4 * NCH))
        epool = ctx.enter_context(tc.tile_pool(name="ew", bufs=4 * NCH))
        hpool = ctx.enter_context(tc.tile_pool(name="h2", bufs=3 * NCH))
        fpool = ctx.enter_context(tc.tile_pool(name="feat", bufs=1))
        opool = ctx.enter_context(tc.tile_pool(name="outs", bufs=1))
        zb_ps = ctx.enter_context(tc.tile_pool(name="zr", bufs=2 * NCH, space="PSUM"))
        nb_xh = 2 if NCH == 1 else NCH
        xh_ps = ctx.enter_context(tc.tile_pool(name="xh", bufs=nb_xh, space="PSUM"))
        rh_ps = ctx.enter_context(tc.tile_pool(name="rh", bufs=nb_xh, space="PSUM"))

        # ---- load weights ----
        w_sb = {}
        for n in WEIGHT_NAMES:
            wt = wpool.tile(list(d_w[n].shape), sdt, tag=f"w_{n}")
            nc.sync.dma_start(wt[:], d_w[n])
            w_sb[n] = wt
        v_sb = {}
        for n in VEC_NAMES:
            vt = wpool.tile(list(d_v[n].shape), DT.float32, tag=f"v_{n}")
            nc.sync.dma_start(vt[:], d_v[n])
            v_sb[n] = vt

        h0_t = wpool.tile([2 * H, BC], sdt, tag="h0t")
        nc.sync.dma_start(h0_t[:], d_h0)
        xin_sb = wpool.tile([2 * F + 1, T * BC], sdt, tag="xin_sb")
        nc.sync.dma_start(xin_sb[:], d_xin)
        zeros_t = wpool.tile([2 * H, BC], sdt, tag="zeros")
        nc.vector.memset(zeros_t[:], 0.0)

        # layer-1 output sequence, one tile per chain so the chains share no
        # tile and stay schedulable independently. Column block s holds
        # [h_fwd(time s); h_bwd(time T-1-s)] for that chain's batch columns.
        seq_t = [seq_pool.tile([2 * H, T * N], sdt, tag=f"seq{c}",
                               name=f"seq{c}")
                 for c in range(NCH)]

        def seq_sl(s, c):
            return seq_t[c][:, s * N: (s + 1) * N]

        feat = fpool.tile([2 * H, BC], sdt, tag="feat")

        rep_ctx = tc.For_i(0, reps, 1) if reps > 1 else None
        if rep_ctx is not None:
            rep_ctx.__enter__()

        def emit_mms(specs):
            """Emit matmuls assigning start/stop per PSUM bank (2KB zero
            region): first matmul into a bank starts the group, last stops."""
            banks = {}
            for i, (out_ap, lhsT, rhs) in enumerate(specs):
                bk = (id(out_ap.tensor), out_ap.offset // 512)
                banks.setdefault(bk, []).append(i)
            for i, (out_ap, lhsT, rhs) in enumerate(specs):
                bk = (id(out_ap.tensor), out_ap.offset // 512)
                nc.tensor.matmul(out_ap, lhsT, rhs,
                                 start=(banks[bk][0] == i),
                                 stop=(banks[bk][-1] == i))

        def gru_step(layer, s, c, x_specs_f, h_prev, h_out):
            """Emit one fused fwd+bwd GRU step for chain c.

            The z-gate weights are negated at prep time, so the PSUM z-half
            holds -z_pre and sigmoid of it yields b = 1-z directly. Blend:
            neg_a = (b-1)*h_prev = -z*h_prev  (Pool, off critical path)
            c     = b*hh
            h     = c - neg_a = (1-z)*hh + z*h_prev
            """
            zrt = zb_ps.tile([2 * H, 2 * N], DT.float32, tag="zr")
            xh_t = xh_ps.tile([2 * H, N], DT.float32, tag="xh")
            xh = xh_t[:]
            rh_t = rh_ps.tile([2 * H, N], DT.float32, tag="rh")
            u = {g: w_sb[f"l{layer}u_{g}"] for g in "zrh"}
            rh = rh_t[:]
            zr_specs = (x_specs_f("z", zrt[:, 0:N])
                        + x_specs_f("r", zrt[:, N: 2 * N])
                        + [(zrt[:, 0:N], _mm(u["z"][:]), _mm(h_prev)),
                           (zrt[:, N: 2 * N], _mm(u["r"][:]), _mm(h_prev))])
            emit_mms(zr_specs)
            # rh before the xh projections: rh gates the critical path (stt t)
            nc.tensor.matmul(rh_t[:], _mm(u["h"][:]), _mm(h_prev),
                             start=True, stop=True)
            xh_specs = x_specs_f("h", xh)
            for i, (out_ap, lhsT, rhs) in enumerate(xh_specs):
                # without inject the last x matmul closes the group
                last = NO_INJECT and i == len(xh_specs) - 1
                nc.tensor.matmul(out_ap, lhsT, rhs, start=(i == 0), stop=last)
            # gates: sg = [b | r], one ACT op unless biases force a split
            sg = spool.tile([2 * H, 2 * N], sdt, tag="sg")
            sgb, sgr = sg[:, 0:N], sg[:, N: 2 * N]
            if layer == 2 and split_l2_sigmoid:
                # bz2 is stored negated (matches negated z weights)
                nc.scalar.activation(sgr, zrt[:, N: 2 * N], AF.Sigmoid,
                                     bias=v_sb["br2"][:])
                nc.scalar.activation(sgb, zrt[:, 0:N], AF.Sigmoid,
                                     bias=v_sb["bz2"][:])
            elif SPLIT_SIG:
                nc.scalar.activation(sgr, zrt[:, N: 2 * N], AF.Sigmoid)
                nc.scalar.activation(sgb, zrt[:, 0:N], AF.Sigmoid)
            else:
                nc.scalar.activation(sg[:], zrt[:], AF.Sigmoid)
            # t = (rh + br_h) * r   (reads PSUM once)
            t = epool.tile([2 * H, N], sdt, tag="t")
            brh = v_sb["br1h" if layer == 1 else "br2h"]
            nc.vector.scalar_tensor_tensor(t[:], rh, brh[:],
                                           sgr, OP.add, OP.mult)
            if NO_INJECT:
                s_t = epool.tile([2 * H, N], sdt, tag="s_t")
                nc.vector.tensor_add(s_t[:], xh, t[:])
                xh = s_t[:]
            else:
                # inject t into the xh accumulation: xh += I @ t (closes group)
                nc.tensor.matmul(xh, _mm(w_sb["ident"][:]), _mm(t[:]),
                                 start=False, stop=True)
            # neg_a = (b-1)*h_prev = -z*h_prev  (Pool, off critical path;
            # Pool has no stt opcode, so two ops: b-1 then multiply)
            bm1 = epool.tile([2 * H, N], sdt, tag="bm1")
            nc.gpsimd.tensor_scalar_sub(bm1[:], sgb, 1.0)
            na = epool.tile([2 * H, N], sdt, tag="na")
            nc.gpsimd.tensor_mul(na[:], bm1[:], h_prev)
            # c = b * act(xh + r*(rh+br_h) + bi_h)
            c_ = epool.tile([2 * H, N], sdt, tag="c_")
            if layer == 1:
                # relu fused: c = max(xh, 0) * b, one DVE op from PSUM
                nc.vector.scalar_tensor_tensor(c_[:], xh, 0.0, sgb,
                                               OP.max, OP.mult)
            else:
                hh = epool.tile([2 * H, N], sdt, tag="hh")
                nc.scalar.activation(hh[:], xh, AF.Tanh,
                                     bias=v_sb["bi2h"][:])
                nc.vector.tensor_mul(c_[:], sgb, hh[:])
            nc.vector.tensor_sub(h_out, c_[:], na[:])

        # ---- layer 1 ----
        for s in range(T):
            for c in range(NCH):
                xt = xin_sb[:, s * BC + c * N: s * BC + c * N + N]

                def l1_x(g, out_ps, _xt=xt):
                    return [(out_ps, _mm(w_sb[f"l1x_{g}"][:]), _mm(_xt))]

                h_prev = (h0_t[:, c * N: c * N + N] if s == 0
                          else seq_sl(s - 1, c))
                gru_step(1, s, c, l1_x, h_prev, seq_sl(s, c))

        # ---- layer 2 ----
        h2_prev = {c: zeros_t[:, c * N: c * N + N] for c in range(NCH)}
        for s in range(T):
            for c in range(NCH):
                x1 = seq_sl(s, c)
                x2 = seq_sl(T - 1 - s, c)

                def l2_x(g, out_ps, _x1=x1, _x2=x2):
                    return [(out_ps, _mm(w_sb[f"l2a_{g}"][:]), _mm(_x1)),
                            (out_ps, _mm(w_sb[f"l2b_{g}"][:]), _mm(_x2))]

                if s == T - 1:
                    h_out = feat[:, c * N: c * N + N]
                else:
                    h2t = hpool.tile([2 * H, N], sdt, tag="h2t")
                    h_out = h2t[:]
                gru_step(2, s, c, l2_x, h2_prev[c], h_out)
                h2_prev[c] = h_out

        # ---- head: relu(feat @ dense_W + b) -> softmax(out_W + b) ----
        ps_d = zb_ps.tile([2 * H, 2 * N], DT.float32, tag="zr")
        nc.tensor.matmul(ps_d[:, 0:BC], _mm(w_sb["dense_w"][:]), _mm(feat[:]),
                         start=True, stop=True)
        h3 = opool.tile([DENSE, BC], sdt, tag="h3")
        nc.scalar.activation(h3[:], ps_d[:, 0:BC], AF.Relu, bias=v_sb["dense_b"][:])

        ps_l = zb_ps.tile([2 * H, 2 * N], DT.float32, tag="zr", name="ps_l")
        nc.tensor.matmul(ps_l[0:P, 0:BC], _mm(w_sb["out_w"][:]), _mm(h3[:]),
                         start=True, stop=True)
        ex = opool.tile([P, BC], DT.float32, tag="ex")
        nc.scalar.activation(ex[:], ps_l[0:P, 0:BC], AF.Exp, bias=v_sb["out_b"][:])

        ones_a = opool.tile([P, 1], DT.float32, tag="ones_a2")
        nc.vector.memset(ones_a[:], 1.0)
        ps_s = zb_ps.tile([2 * H, 2 * N], DT.float32, tag="zr")
        nc.tensor.matmul(ps_s[0:1, 0:BC], ones_a[:], ex[:],
                         start=True, stop=True)
        rin = opool.tile([1, BC], DT.float32, tag="rin")
        nc.vector.reciprocal(rin[:], ps_s[0:1, 0:BC])
        ones_b = opool.tile([1, P], DT.float32, tag="ones_b2")
        nc.vector.memset(ones_b[:], 1.0)
        ps_b = zb_ps.tile([2 * H, 2 * N], DT.float32, tag="zr", name="ps_b")
        nc.tensor.matmul(ps_b[0:P, 0:BC], ones_b[:], rin[:],
                         start=True, stop=True)
        res = opool.tile([P, BC], DT.float32, tag="res")
        nc.vector.tensor_mul(res[:], ex[:], ps_b[0:P, 0:BC])
        nc.sync.dma_start(d_out, res[:])

        if rep_ctx is not None:
            rep_ctx.__exit__(None, None, None)

    nc.finalize()
    return nc


def _gs(g):
    i = "zrh".index(g)
    return slice(i * H, (i + 1) * H)


def prepare_maps(kw):
    """Host-side prep: build per-core input maps (numpy only)."""
    f32 = np.float32
    npdt = _np_dt()
    x = np.asarray(kw["x"], f32)

    wm = {}
    for g in "zrh":
        gs = _gs(g)
        l1x = np.zeros((2 * F + 1, 2 * H), f32)
        l1x[0:F, 0:H] = kw["d1f_W"][:, gs]
        l1x[F:2 * F, H:2 * H] = kw["d1b_W"][:, gs]
        bias_f = kw["d1f_bi"][gs] + (kw["d1f_br"][gs] if g != "h" else 0.0)
        bias_b = kw["d1b_bi"][gs] + (kw["d1b_br"][gs] if g != "h" else 0.0)
        l1x[2 * F, 0:H] = bias_f
        l1x[2 * F, H:2 * H] = bias_b
        wm[f"l1x_{g}"] = l1x

        for lu, uf, ub in ((f"l1u_{g}", kw["d1f_U"], kw["d1b_U"]),
                           (f"l2u_{g}", kw["d2f_U"], kw["d2b_U"])):
            m = np.zeros((2 * H, 2 * H), f32)
            m[0:H, 0:H] = uf[:, gs]
            m[H:2 * H, H:2 * H] = ub[:, gs]
            wm[lu] = m

        a = np.zeros((2 * H, 2 * H), f32)
        a[0:H, 0:H] = kw["d2f_W"][0:H, gs]
        a[H:2 * H, H:2 * H] = kw["d2b_W"][H:2 * H, gs]
        wm[f"l2a_{g}"] = a
        b_ = np.zeros((2 * H, 2 * H), f32)
        b_[0:H, H:2 * H] = kw["d2b_W"][0:H, gs]
        b_[H:2 * H, 0:H] = kw["d2f_W"][H:2 * H, gs]
        wm[f"l2b_{g}"] = b_

    wm["ident"] = np.eye(2 * H, dtype=f32)
    wm["dense_w"] = np.asarray(kw["dense_W"], f32)
    wm["out_w"] = np.asarray(kw["out_W"], f32)

    # negate all z-gate weights: sigmoid(-z_pre) = 1 - z = b directly
    for n in ("l1x_z", "l1u_z", "l2u_z", "l2a_z", "l2b_z"):
        wm[n] = -wm[n]

    vm = {
        "br1h": np.concatenate([kw["d1f_br"][_gs("h")], kw["d1b_br"][_gs("h")]]),
        "br2h": np.concatenate([kw["d2f_br"][_gs("h")], kw["d2b_br"][_gs("h")]]),
        "bi2h": np.concatenate([kw["d2f_bi"][_gs("h")], kw["d2b_bi"][_gs("h")]]),
        # stored negated to match the negated z weights
        "bz2": -np.concatenate([kw["d2f_bi"][_gs("z")] + kw["d2f_br"][_gs("z")],
                                kw["d2b_bi"][_gs("z")] + kw["d2b_br"][_gs("z")]]),
        "br2": np.concatenate([kw["d2f_bi"][_gs("r")] + kw["d2f_br"][_gs("r")],
                               kw["d2b_bi"][_gs("r")] + kw["d2b_br"][_gs("r")]]),
        "dense_b": np.asarray(kw["dense_b"], f32),
        "out_b": np.asarray(kw["out_b"], f32),
        "ones_a": np.ones(P, f32),
        "ones_b": np.ones(P, f32),
    }
    split_l2 = bool(np.any(vm["bz2"]) or np.any(vm["br2"]))

    base = {n: np.ascontiguousarray(w.astype(npdt)) for n, w in wm.items()}
    for n, v in vm.items():
        shape = (1, P) if n == "ones_b" else (P, 1) if n in ("out_b", "ones_a") \
            else (2 * H, 1)
        base[n] = np.ascontiguousarray(np.asarray(v, f32).reshape(shape))

    in_maps = []
    for c in range(NCORES):
        bs = slice(c * BC, (c + 1) * BC)
        xc = x[bs]  # (BC, T, F)
        xin = np.empty((2 * F + 1, T, BC), f32)
        xin[0:F] = xc.transpose(2, 1, 0)
        xin[F:2 * F] = xc[:, ::-1, :].transpose(2, 1, 0)
        xin[2 * F] = 1.0
        xin = xin.reshape(2 * F + 1, T * BC)
        h0 = np.concatenate([np.asarray(kw["h0_fwd"], f32)[bs].T,
                             np.asarray(kw["h0_bwd"], f32)[bs].T], axis=0)
        m = dict(base)
        m["xin"] = np.ascontiguousarray(xin.astype(npdt))
        m["h0"] = np.ascontiguousarray(h0.astype(npdt))
        in_maps.append(m)
    return in_maps, split_l2


_CACHE = {}


def kernel(**inputs) -> np.ndarray:
    in_maps, split_l2 = prepare_maps(inputs)
    key = ("mod", split_l2)
    if key not in _CACHE:
        _CACHE[key] = build_module(split_l2)
    nc = _CACHE[key]
    res = run_bass_kernel_spmd(nc, in_maps, core_ids=list(range(NCORES)))
    outs = [r["out"] for r in res.results]  # each (P, BC)
    full = np.concatenate([o.T for o in outs], axis=0)  # (B, P)
    return np.ascontiguousarray(full.astype(np.float32))



# revision 12
# speedup vs baseline: 1.1131x; 1.1131x over previous
"""Trainium2 Bass kernel: 2-layer bidirectional GRU decoder + dense/softmax head.

Data-parallel over 8 NeuronCores (batch 4096 -> 512 per core). Inside each
core everything runs transposed: partition dim = hidden units with
[fwd(64); bwd(64)] packed to 128 partitions, batch on the free dim.
"""

import os
import sys

sys.path.insert(0, "/opt/trn_rl_repo")

from contextlib import ExitStack

import numpy as np

import concourse.bass as bass
import concourse.bacc as bacc
import concourse.tile as tile
from concourse import mybir
from concourse.bass_utils import run_bass_kernel_spmd

AF = mybir.ActivationFunctionType
OP = mybir.AluOpType
DT = mybir.dt

B, T, F, H, DENSE, P = 4096, 72, 9, 64, 128, 24
NCORES = 8
BC = B // NCORES  # 512 batch per core
G3 = 3 * H

# ---- tuning knobs ----
N_CHUNK = 256        # batch columns per chain (512 = 1 chain, 256 = 2 chains)
STREAM_DT = "bf16"    # "f32" (fp32 storage, fp32r matmuls) or "bf16"
MM_EXACT = False     # True: plain fp32 matmuls (4 cyc/row) for max accuracy
SPLIT_SIG = False    # True: two [128,N] sigmoids (r first); False: one [128,2N]
NO_INJECT = False    # True: s = xh + t on DVE; False: PE identity-inject
INTERLEAVE = True    # True: emit chains phase-interleaved (avoids HOL blocking)


def _np_dt():
    return np.float32 if STREAM_DT == "f32" else np.dtype("bfloat16")


def _mb_dt():
    return DT.float32 if STREAM_DT == "f32" else DT.bfloat16


def _mm(ap):
    """Cast an AP to the matmul dtype (fp32r trick for fp32 streams)."""
    if STREAM_DT == "f32" and not MM_EXACT:
        return ap.bitcast(DT.float32r)
    return ap


WEIGHT_NAMES = (
    ["l1x_z", "l1x_r", "l1x_h"]
    + ["l1u_z", "l1u_r", "l1u_h"]
    + ["l2a_z", "l2a_r", "l2a_h"]
    + ["l2b_z", "l2b_r", "l2b_h"]
    + ["l2u_z", "l2u_r", "l2u_h"]
    + ["ident", "dense_w", "out_w"]
)
VEC_NAMES = ["br1h", "br2h", "bi2h", "bz2", "br2", "dense_b", "out_b",
             "ones_a", "ones_b"]


def build_module(split_l2_sigmoid: bool, reps: int = 1):
    nc = bacc.Bacc("TRN2", target_bir_lowering=False, debug=False)
    sdt = _mb_dt()

    # ---- DRAM I/O ----
    d_xin = nc.dram_tensor("xin", [2 * F + 1, T * BC], sdt, kind="ExternalInput").ap()
    d_h0 = nc.dram_tensor("h0", [2 * H, BC], sdt, kind="ExternalInput").ap()
    d_w = {}
    for n in WEIGHT_NAMES:
        shape = {
            "l1x_z": [2 * F + 1, 2 * H], "l1x_r": [2 * F + 1, 2 * H],
            "l1x_h": [2 * F + 1, 2 * H],
            "dense_w": [2 * H, DENSE], "out_w": [DENSE, P],
        }.get(n, [2 * H, 2 * H])
        d_w[n] = nc.dram_tensor(n, shape, sdt, kind="ExternalInput").ap()
    d_v = {}
    for n in VEC_NAMES:
        shape = {"out_b": [P, 1], "ones_a": [P, 1], "ones_b": [1, P]}.get(n, [2 * H, 1])
        d_v[n] = nc.dram_tensor(n, shape, DT.float32, kind="ExternalInput").ap()
    d_out = nc.dram_tensor("out", [P, BC], DT.float32, kind="ExternalOutput").ap()

    N = N_CHUNK
    NCH = BC // N  # number of chains

    with tile.TileContext(nc) as tc, ExitStack() as ctx:
        wpool = ctx.enter_context(tc.tile_pool(name="weights", bufs=1))
        seq_pool = ctx.enter_context(tc.tile_pool(name="seq", bufs=1))
        spool = ctx.enter_context(tc.tile_pool(name="sig", bufs=4 * NCH))
        epool = ctx.enter_context(tc.tile_pool(name="ew", bufs=4 * NCH))
        hpool = ctx.enter_context(tc.tile_pool(name="h2", bufs=3 * NCH))
        fpool = ctx.enter_context(tc.tile_pool(name="feat", bufs=1))
        opool = ctx.enter_context(tc.tile_pool(name="outs", bufs=1))
        zb_ps = ctx.enter_context(tc.tile_pool(name="zr", bufs=2 * NCH, space="PSUM"))
        nb_xh = 2 if NCH == 1 else NCH
        xh_ps = ctx.enter_context(tc.tile_pool(name="xh", bufs=nb_xh, space="PSUM"))
        rh_ps = ctx.enter_context(tc.tile_pool(name="rh", bufs=nb_xh, space="PSUM"))

        # ---- load weights ----
        w_sb = {}
        for n in WEIGHT_NAMES:
            wt = wpool.tile(list(d_w[n].shape), sdt, tag=f"w_{n}")
            nc.sync.dma_start(wt[:], d_w[n])
            w_sb[n] = wt
        v_sb = {}
        for n in VEC_NAMES:
            vt = wpool.tile(list(d_v[n].shape), DT.float32, tag=f"v_{n}")
            nc.sync.dma_start(vt[:], d_v[n])
            v_sb[n] = vt

        h0_t = wpool.tile([2 * H, BC], sdt, tag="h0t")
        nc.sync.dma_start(h0_t[:], d_h0)
        xin_sb = wpool.tile([2 * F + 1, T * BC], sdt, tag="xin_sb")
        nc.sync.dma_start(xin_sb[:], d_xin)
        zeros_t = wpool.tile([2 * H, BC], sdt, tag="zeros")
        nc.vector.memset(zeros_t[:], 0.0)

        # layer-1 output sequence, one tile per chain so the chains share no
        # tile and stay schedulable independently. Column block s holds
        # [h_fwd(time s); h_bwd(time T-1-s)] for that chain's batch columns.
        seq_t = [seq_pool.tile([2 * H, T * N], sdt, tag=f"seq{c}",
                               name=f"seq{c}")
                 for c in range(NCH)]

        def seq_sl(s, c):
            return seq_t[c][:, s * N: (s + 1) * N]

        feat = fpool.tile([2 * H, BC], sdt, tag="feat")

        rep_ctx = tc.For_i(0, reps, 1) if reps > 1 else None
        if rep_ctx is not None:
            rep_ctx.__enter__()

        def emit_mms(specs):
            """Emit matmuls assigning start/stop per PSUM bank (2KB zero
            region): first matmul into a bank starts the group, last stops."""
            banks = {}
            for i, (out_ap, lhsT, rhs) in enumerate(specs):
                bk = (id(out_ap.tensor), out_ap.offset // 512)
                banks.setdefault(bk, []).append(i)
            for i, (out_ap, lhsT, rhs) in enumerate(specs):
                bk = (id(out_ap.tensor), out_ap.offset // 512)
                nc.tensor.matmul(out_ap, lhsT, rhs,
                                 start=(banks[bk][0] == i),
                                 stop=(banks[bk][-1] == i))

        def gru_step_all(layer, s, chains):
            """Emit one fused fwd+bwd GRU step for all chains, phase by
            phase, so each engine's in-order queue alternates chains and a
            stalled op of one chain never blocks a ready op of the other.

            chains: list of (x_specs_f, h_prev, h_out).

            The z-gate weights are negated at prep time, so the PSUM z-half
            holds -z_pre and sigmoid of it yields b = 1-z directly. Blend:
            neg_a = (b-1)*h_prev = -z*h_prev  (Pool, off critical path)
            c     = b*hh
            h     = c - neg_a = (1-z)*hh + z*h_prev
            """
            nch = len(chains)
            u = {g: w_sb[f"l{layer}u_{g}"] for g in "zrh"}
            brh = v_sb["br1h" if layer == 1 else "br2h"]
            zrt = [zb_ps.tile([2 * H, 2 * N], DT.float32, tag="zr",
                              name=f"zr{c}") for c in range(nch)]
            xh = [xh_ps.tile([2 * H, N], DT.float32, tag="xh",
                             name=f"xh{c}")[:] for c in range(nch)]
            rh = [rh_ps.tile([2 * H, N], DT.float32, tag="rh",
                             name=f"rh{c}")[:] for c in range(nch)]
            sg = [spool.tile([2 * H, 2 * N], sdt, tag="sg",
                             name=f"sg{c}") for c in range(nch)]
            t = [epool.tile([2 * H, N], sdt, tag="t", name=f"t{c}")
                 for c in range(nch)]
            bm1 = [epool.tile([2 * H, N], sdt, tag="bm1", name=f"bm1{c}")
                   for c in range(nch)]
            na = [epool.tile([2 * H, N], sdt, tag="na", name=f"na{c}")
                  for c in range(nch)]
            c_ = [epool.tile([2 * H, N], sdt, tag="c_", name=f"c_{c}")
                  for c in range(nch)]

            def phases():
                if INTERLEAVE:
                    for ph in range(N_PHASES):
                        for c in range(nch):
                            yield ph, c
                else:
                    for c in range(nch):
                        for ph in range(N_PHASES):
                            yield ph, c

            N_PHASES = 8
            for ph, c in phases():
                x_specs_f, h_prev, h_out = chains[c]
                if ph == 0:      # zr matmul group + rh + xh x-parts
                    zr_specs = (x_specs_f("z", zrt[c][:, 0:N])
                                + x_specs_f("r", zrt[c][:, N: 2 * N])
                                + [(zrt[c][:, 0:N], _mm(u["z"][:]), _mm(h_prev)),
                                   (zrt[c][:, N: 2 * N], _mm(u["r"][:]), _mm(h_prev))])
                    emit_mms(zr_specs)
                    nc.tensor.matmul(rh[c], _mm(u["h"][:]), _mm(h_prev),
                                     start=True, stop=True)
                    xh_specs = x_specs_f("h", xh[c])
                    for i, (out_ap, lhsT, rhs) in enumerate(xh_specs):
                        last = NO_INJECT and i == len(xh_specs) - 1
                        nc.tensor.matmul(out_ap, lhsT, rhs,
                                         start=(i == 0), stop=last)
                elif ph == 1:    # sigmoid gates: sg = [b | r]
                    sgb, sgr = sg[c][:, 0:N], sg[c][:, N: 2 * N]
                    if layer == 2 and split_l2_sigmoid:
                        # bz2 is stored negated (matches negated z weights)
                        nc.scalar.activation(sgr, zrt[c][:, N: 2 * N],
                                             AF.Sigmoid, bias=v_sb["br2"][:])
                        nc.scalar.activation(sgb, zrt[c][:, 0:N],
                                             AF.Sigmoid, bias=v_sb["bz2"][:])
                    elif SPLIT_SIG:
                        nc.scalar.activation(sgr, zrt[c][:, N: 2 * N],
                                             AF.Sigmoid)
                        nc.scalar.activation(sgb, zrt[c][:, 0:N], AF.Sigmoid)
                    else:
                        nc.scalar.activation(sg[c][:], zrt[c][:], AF.Sigmoid)
                elif ph == 2:    # t = (rh + br_h) * r
                    nc.vector.scalar_tensor_tensor(
                        t[c][:], rh[c], brh[:], sg[c][:, N: 2 * N],
                        OP.add, OP.mult)
                elif ph == 3:    # xh += t
                    if NO_INJECT:
                        s_t = epool.tile([2 * H, N], sdt, tag="s_t",
                                         name=f"s_t{c}")
                        nc.vector.tensor_add(s_t[:], xh[c], t[c][:])
                        xh[c] = s_t[:]
                    else:
                        nc.tensor.matmul(xh[c], _mm(w_sb["ident"][:]),
                                         _mm(t[c][:]), start=False, stop=True)
                elif ph == 4:    # neg_a = (b-1)*h_prev on Pool (two ops)
                    nc.gpsimd.tensor_scalar_sub(bm1[c][:], sg[c][:, 0:N], 1.0)
                elif ph == 5:
                    nc.gpsimd.tensor_mul(na[c][:], bm1[c][:], h_prev)
                elif ph == 6:    # c = b * act(xh_total + bi_h)
                    if layer == 1:
                        nc.vector.scalar_tensor_tensor(
                            c_[c][:], xh[c], 0.0, sg[c][:, 0:N],
                            OP.max, OP.mult)
                    else:
                        hh = epool.tile([2 * H, N], sdt, tag="hh")
                        nc.scalar.activation(hh[:], xh[c], AF.Tanh,
                                             bias=v_sb["bi2h"][:])
                        nc.vector.tensor_mul(c_[c][:], sg[c][:, 0:N], hh[:])
                elif ph == 7:    # h = c - neg_a
                    nc.vector.tensor_sub(h_out, c_[c][:], na[c][:])

        # ---- layer 1 ----
        for s in range(T):
            chains = []
            for c in range(NCH):
                xt = xin_sb[:, s * BC + c * N: s * BC + c * N + N]

                def l1_x(g, out_ps, _xt=xt):
                    return [(out_ps, _mm(w_sb[f"l1x_{g}"][:]), _mm(_xt))]

                h_prev = (h0_t[:, c * N: c * N + N] if s == 0
                          else seq_sl(s - 1, c))
                chains.append((l1_x, h_prev, seq_sl(s, c)))
            gru_step_all(1, s, chains)

        # ---- layer 2 ----
        h2_prev = {c: zeros_t[:, c * N: c * N + N] for c in range(NCH)}
        for s in range(T):
            chains = []
            h_outs = {}
            for c in range(NCH):
                x1 = seq_sl(s, c)
                x2 = seq_sl(T - 1 - s, c)

                def l2_x(g, out_ps, _x1=x1, _x2=x2):
                    return [(out_ps, _mm(w_sb[f"l2a_{g}"][:]), _mm(_x1)),
                            (out_ps, _mm(w_sb[f"l2b_{g}"][:]), _mm(_x2))]

                if s == T - 1:
                    h_out = feat[:, c * N: c * N + N]
                else:
                    h2t = hpool.tile([2 * H, N], sdt, tag="h2t")
                    h_out = h2t[:]
                h_outs[c] = h_out
                chains.append((l2_x, h2_prev[c], h_out))
            gru_step_all(2, s, chains)
            for c in range(NCH):
                h2_prev[c] = h_outs[c]

        # ---- head: relu(feat @ dense_W + b) -> softmax(out_W + b) ----
        ps_d = zb_ps.tile([2 * H, 2 * N], DT.float32, tag="zr")
        nc.tensor.matmul(ps_d[:, 0:BC], _mm(w_sb["dense_w"][:]), _mm(feat[:]),
                         start=True, stop=True)
        h3 = opool.tile([DENSE, BC], sdt, tag="h3")
        nc.scalar.activation(h3[:], ps_d[:, 0:BC], AF.Relu, bias=v_sb["dense_b"][:])

        ps_l = zb_ps.tile([2 * H, 2 * N], DT.float32, tag="zr", name="ps_l")
        nc.tensor.matmul(ps_l[0:P, 0:BC], _mm(w_sb["out_w"][:]), _mm(h3[:]),
                         start=True, stop=True)
        ex = opool.tile([P, BC], DT.float32, tag="ex")
        nc.scalar.activation(ex[:], ps_l[0:P, 0:BC], AF.Exp, bias=v_sb["out_b"][:])

        ones_a = opool.tile([P, 1], DT.float32, tag="ones_a2")
        nc.vector.memset(ones_a[:], 1.0)
        ps_s = zb_ps.tile([2 * H, 2 * N], DT.float32, tag="zr")
        nc.tensor.matmul(ps_s[0:1, 0:BC], ones_a[:], ex[:],
                         start=True, stop=True)
        rin = opool.tile([1, BC], DT.float32, tag="rin")
        nc.vector.reciprocal(rin[:], ps_s[0:1, 0:BC])
        ones_b = opool.tile([1, P], DT.float32, tag="ones_b2")
        nc.vector.memset(ones_b[:], 1.0)
        ps_b = zb_ps.tile([2 * H, 2 * N], DT.float32, tag="zr", name="ps_b")
        nc.tensor.matmul(ps_b[0:P, 0:BC], ones_b[:], rin[:],
                         start=True, stop=True)
        res = opool.tile([P, BC], DT.float32, tag="res")
        nc.vector.tensor_mul(res[:], ex[:], ps_b[0:P, 0:BC])
        nc.sync.dma_start(d_out, res[:])

        if rep_ctx is not None:
            rep_ctx.__exit__(None, None, None)

    nc.finalize()
    return nc


def _gs(g):
    i = "zrh".index(g)
    return slice(i * H, (i + 1) * H)


def prepare_maps(kw):
    """Host-side prep: build per-core input maps (numpy only)."""
    f32 = np.float32
    npdt = _np_dt()
    x = np.asarray(kw["x"], f32)

    wm = {}
    for g in "zrh":
        gs = _gs(g)
        l1x = np.zeros((2 * F + 1, 2 * H), f32)
        l1x[0:F, 0:H] = kw["d1f_W"][:, gs]
        l1x[F:2 * F, H:2 * H] = kw["d1b_W"][:, gs]
        bias_f = kw["d1f_bi"][gs] + (kw["d1f_br"][gs] if g != "h" else 0.0)
        bias_b = kw["d1b_bi"][gs] + (kw["d1b_br"][gs] if g != "h" else 0.0)
        l1x[2 * F, 0:H] = bias_f
        l1x[2 * F, H:2 * H] = bias_b
        wm[f"l1x_{g}"] = l1x

        for lu, uf, ub in ((f"l1u_{g}", kw["d1f_U"], kw["d1b_U"]),
                           (f"l2u_{g}", kw["d2f_U"], kw["d2b_U"])):
            m = np.zeros((2 * H, 2 * H), f32)
            m[0:H, 0:H] = uf[:, gs]
            m[H:2 * H, H:2 * H] = ub[:, gs]
            wm[lu] = m

        a = np.zeros((2 * H, 2 * H), f32)
        a[0:H, 0:H] = kw["d2f_W"][0:H, gs]
        a[H:2 * H, H:2 * H] = kw["d2b_W"][H:2 * H, gs]
        wm[f"l2a_{g}"] = a
        b_ = np.zeros((2 * H, 2 * H), f32)
        b_[0:H, H:2 * H] = kw["d2b_W"][0:H, gs]
        b_[H:2 * H, 0:H] = kw["d2f_W"][H:2 * H, gs]
        wm[f"l2b_{g}"] = b_

    wm["ident"] = np.eye(2 * H, dtype=f32)
    wm["dense_w"] = np.asarray(kw["dense_W"], f32)
    wm["out_w"] = np.asarray(kw["out_W"], f32)

    # negate all z-gate weights: sigmoid(-z_pre) = 1 - z = b directly
    for n in ("l1x_z", "l1u_z", "l2u_z", "l2a_z", "l2b_z"):
        wm[n] = -wm[n]

    vm = {
        "br1h": np.concatenate([kw["d1f_br"][_gs("h")], kw["d1b_br"][_gs("h")]]),
        "br2h": np.concatenate([kw["d2f_br"][_gs("h")], kw["d2b_br"][_gs("h")]]),
        "bi2h": np.concatenate([kw["d2f_bi"][_gs("h")], kw["d2b_bi"][_gs("h")]]),
        # stored negated to match the negated z weights
        "bz2": -np.concatenate([kw["d2f_bi"][_gs("z")] + kw["d2f_br"][_gs("z")],
                                kw["d2b_bi"][_gs("z")] + kw["d2b_br"][_gs("z")]]),
        "br2": np.concatenate([kw["d2f_bi"][_gs("r")] + kw["d2f_br"][_gs("r")],
                               kw["d2b_bi"][_gs("r")] + kw["d2b_br"][_gs("r")]]),
        "dense_b": np.asarray(kw["dense_b"], f32),
        "out_b": np.asarray(kw["out_b"], f32),
        "ones_a": np.ones(P, f32),
        "ones_b": np.ones(P, f32),
    }
    split_l2 = bool(np.any(vm["bz2"]) or np.any(vm["br2"]))

    base = {n: np.ascontiguousarray(w.astype(npdt)) for n, w in wm.items()}
    for n, v in vm.items():
        shape = (1, P) if n == "ones_b" else (P, 1) if n in ("out_b", "ones_a") \
            else (2 * H, 1)
        base[n] = np.ascontiguousarray(np.asarray(v, f32).reshape(shape))

    in_maps = []
    for c in range(NCORES):
        bs = slice(c * BC, (c + 1) * BC)
        xc = x[bs]  # (BC, T, F)
        xin = np.empty((2 * F + 1, T, BC), f32)
        xin[0:F] = xc.transpose(2, 1, 0)
        xin[F:2 * F] = xc[:, ::-1, :].transpose(2, 1, 0)
        xin[2 * F] = 1.0
        xin = xin.reshape(2 * F + 1, T * BC)
        h0 = np.concatenate([np.asarray(kw["h0_fwd"], f32)[bs].T,
                             np.asarray(kw["h0_bwd"], f32)[bs].T], axis=0)
        m = dict(base)
        m["xin"] = np.ascontiguousarray(xin.astype(npdt))
        m["h0"] = np.ascontiguousarray(h0.astype(npdt))
        in_maps.append(m)
    return in_maps, split_l2


_CACHE = {}


def kernel(**inputs) -> np.ndarray:
    in_maps, split_l2 = prepare_maps(inputs)
    key = ("mod", split_l2)
    if key not in _CACHE:
        _CACHE[key] = build_module(split_l2)
    nc = _CACHE[key]
    res = run_bass_kernel_spmd(nc, in_maps, core_ids=list(range(NCORES)))
    outs = [r["out"] for r in res.results]  # each (P, BC)
    full = np.concatenate([o.T for o in outs], axis=0)  # (B, P)
    return np.ascontiguousarray(full.astype(np.float32))



# revision 18
# speedup vs baseline: 2.1325x; 1.9158x over previous
"""Trainium2 Bass kernel: 2-layer bidirectional GRU decoder + dense/softmax head.

Data-parallel over 8 NeuronCores (batch 4096 -> 512 per core). Inside each
core everything runs transposed: partition dim = hidden units with
[fwd(64); bwd(64)] packed to 128 partitions, batch on the free dim.
"""

import os
import sys

sys.path.insert(0, "/opt/trn_rl_repo")

from contextlib import ExitStack

import numpy as np

import concourse.bass as bass
import concourse.bacc as bacc
import concourse.tile as tile
from concourse import mybir
from concourse.bass_utils import run_bass_kernel_spmd

AF = mybir.ActivationFunctionType
OP = mybir.AluOpType
DT = mybir.dt

B, T, F, H, DENSE, P = 4096, 72, 9, 64, 128, 24
NCORES = 8
BC = B // NCORES  # 512 batch per core
G3 = 3 * H

# ---- tuning knobs ----
N_CHUNK = 256        # batch columns per chain (512 = 1 chain, 256 = 2 chains)
STREAM_DT = "bf16"    # "f32" (fp32 storage, fp32r matmuls) or "bf16"
MM_EXACT = False     # True: plain fp32 matmuls (4 cyc/row) for max accuracy
SPLIT_SIG = False    # True: two [128,N] sigmoids (r first); False: one [128,2N]
NO_INJECT = False    # True: s = xh + t on DVE; False: PE identity-inject
INTERLEAVE = True    # True: emit chains phase-interleaved (avoids HOL blocking)
USE_POOL = False     # keep GPSIMD idle: its SBUF port contends with DVE


def _np_dt():
    return np.float32 if STREAM_DT == "f32" else np.dtype("bfloat16")


def _mb_dt():
    return DT.float32 if STREAM_DT == "f32" else DT.bfloat16


def _mm(ap):
    """Cast an AP to the matmul dtype (fp32r trick for fp32 streams)."""
    if STREAM_DT == "f32" and not MM_EXACT:
        return ap.bitcast(DT.float32r)
    return ap


WEIGHT_NAMES = (
    ["l1x_z", "l1x_r", "l1x_h"]
    + ["l1u_z", "l1u_r", "l1u_h"]
    + ["l2a_z", "l2a_r", "l2a_h"]
    + ["l2b_z", "l2b_r", "l2b_h"]
    + ["l2u_z", "l2u_r", "l2u_h"]
    + ["ident", "dense_w", "out_w"]
)
VEC_NAMES = ["br1h", "br2h", "bi2h", "bz2", "br2", "dense_b", "out_b",
             "ones_a", "ones_b"]


def build_module(split_l2_sigmoid: bool, reps: int = 1):
    nc = bacc.Bacc("TRN2", target_bir_lowering=False, debug=False)
    sdt = _mb_dt()

    # ---- DRAM I/O ----
    d_xin = nc.dram_tensor("xin", [2 * F + 1, T * BC], sdt, kind="ExternalInput").ap()
    d_h0 = nc.dram_tensor("h0", [2 * H, BC], sdt, kind="ExternalInput").ap()
    d_w = {}
    for n in WEIGHT_NAMES:
        shape = {
            "l1x_z": [2 * F + 1, 2 * H], "l1x_r": [2 * F + 1, 2 * H],
            "l1x_h": [2 * F + 1, 2 * H],
            "dense_w": [2 * H, DENSE], "out_w": [DENSE, P],
        }.get(n, [2 * H, 2 * H])
        d_w[n] = nc.dram_tensor(n, shape, sdt, kind="ExternalInput").ap()
    d_v = {}
    for n in VEC_NAMES:
        shape = {"out_b": [P, 1], "ones_a": [P, 1], "ones_b": [1, P]}.get(n, [2 * H, 1])
        d_v[n] = nc.dram_tensor(n, shape, DT.float32, kind="ExternalInput").ap()
    d_out = nc.dram_tensor("out", [P, BC], DT.float32, kind="ExternalOutput").ap()

    N = N_CHUNK
    NCH = BC // N  # number of chains

    with tile.TileContext(nc) as tc, ExitStack() as ctx:
        wpool = ctx.enter_context(tc.tile_pool(name="weights", bufs=1))
        seq_pool = ctx.enter_context(tc.tile_pool(name="seq", bufs=1))
        spool = ctx.enter_context(tc.tile_pool(name="sig", bufs=4 * NCH))
        epool = ctx.enter_context(tc.tile_pool(name="ew", bufs=4 * NCH))
        hpool = ctx.enter_context(tc.tile_pool(name="h2", bufs=3 * NCH))
        fpool = ctx.enter_context(tc.tile_pool(name="feat", bufs=1))
        opool = ctx.enter_context(tc.tile_pool(name="outs", bufs=1))
        nb_zb = 2 * NCH if 2 * N >= BC else NCH
        zb_ps = ctx.enter_context(tc.tile_pool(name="zr", bufs=nb_zb, space="PSUM"))
        nb_xh = 2 if NCH == 1 else NCH
        xh_ps = ctx.enter_context(tc.tile_pool(name="xh", bufs=nb_xh, space="PSUM"))
        rh_ps = ctx.enter_context(tc.tile_pool(name="rh", bufs=nb_xh, space="PSUM"))
        if 2 * N < BC:
            # head tiles need [2H, BC]; zr tiles are too narrow to reuse
            hd_ps = ctx.enter_context(
                tc.tile_pool(name="hd", bufs=2, space="PSUM"))

        # ---- load weights ----
        w_sb = {}
        for n in WEIGHT_NAMES:
            wt = wpool.tile(list(d_w[n].shape), sdt, tag=f"w_{n}")
            nc.sync.dma_start(wt[:], d_w[n])
            w_sb[n] = wt
        v_sb = {}
        for n in VEC_NAMES:
            vt = wpool.tile(list(d_v[n].shape), DT.float32, tag=f"v_{n}")
            nc.sync.dma_start(vt[:], d_v[n])
            v_sb[n] = vt

        h0_t = wpool.tile([2 * H, BC], sdt, tag="h0t")
        nc.sync.dma_start(h0_t[:], d_h0)
        xin_sb = wpool.tile([2 * F + 1, T * BC], sdt, tag="xin_sb")
        nc.sync.dma_start(xin_sb[:], d_xin)
        zeros_t = wpool.tile([2 * H, BC], sdt, tag="zeros")
        nc.vector.memset(zeros_t[:], 0.0)

        # layer-1 output sequence, one tile per chain so the chains share no
        # tile and stay schedulable independently. Column block s holds
        # [h_fwd(time s); h_bwd(time T-1-s)] for that chain's batch columns.
        seq_t = [seq_pool.tile([2 * H, T * N], sdt, tag=f"seq{c}",
                               name=f"seq{c}")
                 for c in range(NCH)]

        def seq_sl(s, c):
            return seq_t[c][:, s * N: (s + 1) * N]

        feat = fpool.tile([2 * H, BC], sdt, tag="feat")

        rep_ctx = tc.For_i(0, reps, 1) if reps > 1 else None
        if rep_ctx is not None:
            rep_ctx.__enter__()

        def emit_mms(specs):
            """Emit matmuls assigning start/stop per PSUM bank (2KB zero
            region): first matmul into a bank starts the group, last stops."""
            banks = {}
            for i, (out_ap, lhsT, rhs) in enumerate(specs):
                bk = (id(out_ap.tensor), out_ap.offset // 512)
                banks.setdefault(bk, []).append(i)
            for i, (out_ap, lhsT, rhs) in enumerate(specs):
                bk = (id(out_ap.tensor), out_ap.offset // 512)
                nc.tensor.matmul(out_ap, lhsT, rhs,
                                 start=(banks[bk][0] == i),
                                 stop=(banks[bk][-1] == i))

        def gru_step_all(layer, s, chains):
            """Emit one fused fwd+bwd GRU step for all chains, phase by
            phase, so each engine's in-order queue alternates chains and a
            stalled op of one chain never blocks a ready op of the other.

            chains: list of (x_specs_f, h_prev, h_out).

            The z-gate weights are negated at prep time, so the PSUM z-half
            holds -z_pre and sigmoid of it yields b = 1-z directly. Blend:
            neg_a = (b-1)*h_prev = -z*h_prev  (Pool, off critical path)
            c     = b*hh
            h     = c - neg_a = (1-z)*hh + z*h_prev
            """
            nch = len(chains)
            u = {g: w_sb[f"l{layer}u_{g}"] for g in "zrh"}
            brh = v_sb["br1h" if layer == 1 else "br2h"]
            zrt = [zb_ps.tile([2 * H, 2 * N], DT.float32, tag="zr",
                              name=f"zr{c}") for c in range(nch)]
            xh = [xh_ps.tile([2 * H, N], DT.float32, tag="xh",
                             name=f"xh{c}")[:] for c in range(nch)]
            rh = [rh_ps.tile([2 * H, N], DT.float32, tag="rh",
                             name=f"rh{c}")[:] for c in range(nch)]
            sg = [spool.tile([2 * H, 2 * N], sdt, tag="sg",
                             name=f"sg{c}") for c in range(nch)]
            t = [epool.tile([2 * H, N], sdt, tag="t", name=f"t{c}")
                 for c in range(nch)]
            bm1 = [epool.tile([2 * H, N], sdt, tag="bm1", name=f"bm1{c}")
                   for c in range(nch)]
            na = [epool.tile([2 * H, N], sdt, tag="na", name=f"na{c}")
                  for c in range(nch)]
            c_ = [epool.tile([2 * H, N], sdt, tag="c_", name=f"c_{c}")
                  for c in range(nch)]

            def phases():
                if INTERLEAVE:
                    for ph in range(N_PHASES):
                        for c in range(nch):
                            yield ph, c
                else:
                    for c in range(nch):
                        for ph in range(N_PHASES):
                            yield ph, c

            N_PHASES = 8
            for ph, c in phases():
                x_specs_f, h_prev, h_out = chains[c]
                if ph == 0:      # zr matmul group + rh + xh x-parts
                    zr_specs = (x_specs_f("z", zrt[c][:, 0:N])
                                + x_specs_f("r", zrt[c][:, N: 2 * N])
                                + [(zrt[c][:, 0:N], _mm(u["z"][:]), _mm(h_prev)),
                                   (zrt[c][:, N: 2 * N], _mm(u["r"][:]), _mm(h_prev))])
                    emit_mms(zr_specs)
                    nc.tensor.matmul(rh[c], _mm(u["h"][:]), _mm(h_prev),
                                     start=True, stop=True)
                    xh_specs = x_specs_f("h", xh[c])
                    for i, (out_ap, lhsT, rhs) in enumerate(xh_specs):
                        last = NO_INJECT and i == len(xh_specs) - 1
                        nc.tensor.matmul(out_ap, lhsT, rhs,
                                         start=(i == 0), stop=last)
                elif ph == 1:    # sigmoid gates: sg = [b | r]
                    sgb, sgr = sg[c][:, 0:N], sg[c][:, N: 2 * N]
                    if layer == 2 and split_l2_sigmoid:
                        # bz2 is stored negated (matches negated z weights)
                        nc.scalar.activation(sgr, zrt[c][:, N: 2 * N],
                                             AF.Sigmoid, bias=v_sb["br2"][:])
                        nc.scalar.activation(sgb, zrt[c][:, 0:N],
                                             AF.Sigmoid, bias=v_sb["bz2"][:])
                    elif SPLIT_SIG:
                        nc.scalar.activation(sgr, zrt[c][:, N: 2 * N],
                                             AF.Sigmoid)
                        nc.scalar.activation(sgb, zrt[c][:, 0:N], AF.Sigmoid)
                    else:
                        nc.scalar.activation(sg[c][:], zrt[c][:], AF.Sigmoid)
                elif ph == 2:    # t = (rh + br_h) * r
                    nc.vector.scalar_tensor_tensor(
                        t[c][:], rh[c], brh[:], sg[c][:, N: 2 * N],
                        OP.add, OP.mult)
                elif ph == 3:    # xh += t
                    if NO_INJECT:
                        s_t = epool.tile([2 * H, N], sdt, tag="s_t",
                                         name=f"s_t{c}")
                        nc.vector.tensor_add(s_t[:], xh[c], t[c][:])
                        xh[c] = s_t[:]
                    else:
                        nc.tensor.matmul(xh[c], _mm(w_sb["ident"][:]),
                                         _mm(t[c][:]), start=False, stop=True)
                elif ph == 4:    # neg_a = (b-1)*h_prev
                    if USE_POOL:  # Pool lacks stt: two ops, off critical path
                        nc.gpsimd.tensor_scalar_sub(bm1[c][:], sg[c][:, 0:N],
                                                    1.0)
                elif ph == 5:
                    if USE_POOL:
                        nc.gpsimd.tensor_mul(na[c][:], bm1[c][:], h_prev)
                    else:
                        nc.vector.scalar_tensor_tensor(
                            na[c][:], sg[c][:, 0:N], 1.0, h_prev,
                            OP.subtract, OP.mult)
                elif ph == 6:    # c = b * act(xh_total + bi_h)
                    if layer == 1:
                        nc.vector.scalar_tensor_tensor(
                            c_[c][:], xh[c], 0.0, sg[c][:, 0:N],
                            OP.max, OP.mult)
                    else:
                        hh = epool.tile([2 * H, N], sdt, tag="hh")
                        nc.scalar.activation(hh[:], xh[c], AF.Tanh,
                                             bias=v_sb["bi2h"][:])
                        nc.vector.tensor_mul(c_[c][:], sg[c][:, 0:N], hh[:])
                elif ph == 7:    # h = c - neg_a
                    nc.vector.tensor_sub(h_out, c_[c][:], na[c][:])

        # ---- layer 1 ----
        for s in range(T):
            chains = []
            for c in range(NCH):
                xt = xin_sb[:, s * BC + c * N: s * BC + c * N + N]

                def l1_x(g, out_ps, _xt=xt):
                    return [(out_ps, _mm(w_sb[f"l1x_{g}"][:]), _mm(_xt))]

                h_prev = (h0_t[:, c * N: c * N + N] if s == 0
                          else seq_sl(s - 1, c))
                chains.append((l1_x, h_prev, seq_sl(s, c)))
            gru_step_all(1, s, chains)

        # ---- layer 2 ----
        h2_prev = {c: zeros_t[:, c * N: c * N + N] for c in range(NCH)}
        for s in range(T):
            chains = []
            h_outs = {}
            for c in range(NCH):
                x1 = seq_sl(s, c)
                x2 = seq_sl(T - 1 - s, c)

                def l2_x(g, out_ps, _x1=x1, _x2=x2):
                    return [(out_ps, _mm(w_sb[f"l2a_{g}"][:]), _mm(_x1)),
                            (out_ps, _mm(w_sb[f"l2b_{g}"][:]), _mm(_x2))]

                if s == T - 1:
                    h_out = feat[:, c * N: c * N + N]
                else:
                    h2t = hpool.tile([2 * H, N], sdt, tag="h2t")
                    h_out = h2t[:]
                h_outs[c] = h_out
                chains.append((l2_x, h2_prev[c], h_out))
            gru_step_all(2, s, chains)
            for c in range(NCH):
                h2_prev[c] = h_outs[c]

        # ---- head: relu(feat @ dense_W + b) -> softmax(out_W + b) ----
        def head_tile(name=None):
            if 2 * N < BC:
                return hd_ps.tile([2 * H, BC], DT.float32, tag="hd",
                                  name=name or "hd_t")
            return zb_ps.tile([2 * H, 2 * N], DT.float32, tag="zr",
                              name=name or "hd_t")

        ps_d = head_tile()
        nc.tensor.matmul(ps_d[:, 0:BC], _mm(w_sb["dense_w"][:]), _mm(feat[:]),
                         start=True, stop=True)
        h3 = opool.tile([DENSE, BC], sdt, tag="h3")
        nc.scalar.activation(h3[:], ps_d[:, 0:BC], AF.Relu, bias=v_sb["dense_b"][:])

        ps_l = head_tile("ps_l")
        nc.tensor.matmul(ps_l[0:P, 0:BC], _mm(w_sb["out_w"][:]), _mm(h3[:]),
                         start=True, stop=True)
        ex = opool.tile([P, BC], DT.float32, tag="ex")
        nc.scalar.activation(ex[:], ps_l[0:P, 0:BC], AF.Exp, bias=v_sb["out_b"][:])

        ones_a = opool.tile([P, 1], DT.float32, tag="ones_a2")
        nc.vector.memset(ones_a[:], 1.0)
        ps_s = head_tile("ps_s")
        nc.tensor.matmul(ps_s[0:1, 0:BC], ones_a[:], ex[:],
                         start=True, stop=True)
        rin = opool.tile([1, BC], DT.float32, tag="rin")
        nc.vector.reciprocal(rin[:], ps_s[0:1, 0:BC])
        ones_b = opool.tile([1, P], DT.float32, tag="ones_b2")
        nc.vector.memset(ones_b[:], 1.0)
        ps_b = head_tile("ps_b")
        nc.tensor.matmul(ps_b[0:P, 0:BC], ones_b[:], rin[:],
                         start=True, stop=True)
        res = opool.tile([P, BC], DT.float32, tag="res")
        nc.vector.tensor_mul(res[:], ex[:], ps_b[0:P, 0:BC])
        nc.sync.dma_start(d_out, res[:])

        if rep_ctx is not None:
            rep_ctx.__exit__(None, None, None)

    nc.finalize()
    return nc


def _gs(g):
    i = "zrh".index(g)
    return slice(i * H, (i + 1) * H)


def prepare_maps(kw):
    """Host-side prep: build per-core input maps (numpy only)."""
    f32 = np.float32
    npdt = _np_dt()
    x = np.asarray(kw["x"], f32)

    wm = {}
    for g in "zrh":
        gs = _gs(g)
        l1x = np.zeros((2 * F + 1, 2 * H), f32)
        l1x[0:F, 0:H] = kw["d1f_W"][:, gs]
        l1x[F:2 * F, H:2 * H] = kw["d1b_W"][:, gs]
        bias_f = kw["d1f_bi"][gs] + (kw["d1f_br"][gs] if g != "h" else 0.0)
        bias_b = kw["d1b_bi"][gs] + (kw["d1b_br"][gs] if g != "h" else 0.0)
        l1x[2 * F, 0:H] = bias_f
        l1x[2 * F, H:2 * H] = bias_b
        wm[f"l1x_{g}"] = l1x

        for lu, uf, ub in ((f"l1u_{g}", kw["d1f_U"], kw["d1b_U"]),
                           (f"l2u_{g}", kw["d2f_U"], kw["d2b_U"])):
            m = np.zeros((2 * H, 2 * H), f32)
            m[0:H, 0:H] = uf[:, gs]
            m[H:2 * H, H:2 * H] = ub[:, gs]
            wm[lu] = m

        a = np.zeros((2 * H, 2 * H), f32)
        a[0:H, 0:H] = kw["d2f_W"][0:H, gs]
        a[H:2 * H, H:2 * H] = kw["d2b_W"][H:2 * H, gs]
        wm[f"l2a_{g}"] = a
        b_ = np.zeros((2 * H, 2 * H), f32)
        b_[0:H, H:2 * H] = kw["d2b_W"][0:H, gs]
        b_[H:2 * H, 0:H] = kw["d2f_W"][H:2 * H, gs]
        wm[f"l2b_{g}"] = b_

    wm["ident"] = np.eye(2 * H, dtype=f32)
    wm["dense_w"] = np.asarray(kw["dense_W"], f32)
    wm["out_w"] = np.asarray(kw["out_W"], f32)

    # negate all z-gate weights: sigmoid(-z_pre) = 1 - z = b directly
    for n in ("l1x_z", "l1u_z", "l2u_z", "l2a_z", "l2b_z"):
        wm[n] = -wm[n]

    vm = {
        "br1h": np.concatenate([kw["d1f_br"][_gs("h")], kw["d1b_br"][_gs("h")]]),
        "br2h": np.concatenate([kw["d2f_br"][_gs("h")], kw["d2b_br"][_gs("h")]]),
        "bi2h": np.concatenate([kw["d2f_bi"][_gs("h")], kw["d2b_bi"][_gs("h")]]),
        # stored negated to match the negated z weights
        "bz2": -np.concatenate([kw["d2f_bi"][_gs("z")] + kw["d2f_br"][_gs("z")],
                                kw["d2b_bi"][_gs("z")] + kw["d2b_br"][_gs("z")]]),
        "br2": np.concatenate([kw["d2f_bi"][_gs("r")] + kw["d2f_br"][_gs("r")],
                               kw["d2b_bi"][_gs("r")] + kw["d2b_br"][_gs("r")]]),
        "dense_b": np.asarray(kw["dense_b"], f32),
        "out_b": np.asarray(kw["out_b"], f32),
        "ones_a": np.ones(P, f32),
        "ones_b": np.ones(P, f32),
    }
    split_l2 = bool(np.any(vm["bz2"]) or np.any(vm["br2"]))

    base = {n: np.ascontiguousarray(w.astype(npdt)) for n, w in wm.items()}
    for n, v in vm.items():
        shape = (1, P) if n == "ones_b" else (P, 1) if n in ("out_b", "ones_a") \
            else (2 * H, 1)
        base[n] = np.ascontiguousarray(np.asarray(v, f32).reshape(shape))

    in_maps = []
    for c in range(NCORES):
        bs = slice(c * BC, (c + 1) * BC)
        xc = x[bs]  # (BC, T, F)
        xin = np.empty((2 * F + 1, T, BC), f32)
        xin[0:F] = xc.transpose(2, 1, 0)
        xin[F:2 * F] = xc[:, ::-1, :].transpose(2, 1, 0)
        xin[2 * F] = 1.0
        xin = xin.reshape(2 * F + 1, T * BC)
        h0 = np.concatenate([np.asarray(kw["h0_fwd"], f32)[bs].T,
                             np.asarray(kw["h0_bwd"], f32)[bs].T], axis=0)
        m = dict(base)
        m["xin"] = np.ascontiguousarray(xin.astype(npdt))
        m["h0"] = np.ascontiguousarray(h0.astype(npdt))
        in_maps.append(m)
    return in_maps, split_l2


_CACHE = {}


def kernel(**inputs) -> np.ndarray:
    in_maps, split_l2 = prepare_maps(inputs)
    key = ("mod", split_l2)
    if key not in _CACHE:
        _CACHE[key] = build_module(split_l2)
    nc = _CACHE[key]
    res = run_bass_kernel_spmd(nc, in_maps, core_ids=list(range(NCORES)))
    outs = [r["out"] for r in res.results]  # each (P, BC)
    full = np.concatenate([o.T for o in outs], axis=0)  # (B, P)
    return np.ascontiguousarray(full.astype(np.float32))

